# revision 22
# baseline (speedup 1.0000x reference)
"""Trainium2 Bass kernel for DeepLACForests (8-core data-parallel over batch).

Computation (matches the reference nn.Module):
  feats = relu(X @ W1 + b1)                  [B, 128]
  prediction = feats @ W2 + b2               [B, 10]
  dec = sigmoid(einsum('bd,tid->tib', feats, nodeW) + nodeb)   [16, 255, B]
  all_results = level-wise prefix product down a depth-9 heap  [16, 511, B]

Sharding: data-parallel over batch (1024 rows/core x 8 cores); encoder and
node weights (~2MB) replicated. On-device layout keeps batch on SBUF
partitions and (tree, node) on the free dimension so every DMA is fully
contiguous; the host pre-transposes X / nodeW and untransposes the result.

Matmuls run as bf16 hi/lo split-precision (x = hi + lo exactly, with
x @ w ~= hi@whi + lo@whi + hi@wlo, ~2^-17 relative error): fp32 matmuls
stream at half rate on the PE, so three bf16 passes beat one fp32 pass
while keeping near-fp32 accuracy.

DMA discipline: inputs arrive in a handful of large multi-dim DMAs issued
from ScalarE's HW-DGE ring; result tiles stream out per tree-half (16KB
contiguous per partition) from SyncE's ring, overlapping the product chain.
"""

from contextlib import ExitStack

import ml_dtypes
import numpy as np

import concourse.bass as bass
import concourse.tile as tile
from concourse import bacc, mybir
from concourse.bass_utils import run_bass_kernel_spmd
from concourse.tile_rust import add_dep_helper

F32 = mybir.dt.float32
BF16 = mybir.dt.bfloat16
AF = mybir.ActivationFunctionType

N_CORES = 8
BATCH = 8192
BC = BATCH // N_CORES          # 1024 batch rows per core
NB = BC // 128                 # 8 partition tiles per core
IN_DIM = 784
KPAD = 896                     # 7 * 128: zero-padded contraction dim
NK = KPAD // 128               # 7 uniform K chunks
HID = 128
CLS = 10
ENSEMBLE = 16
HALF_T = ENSEMBLE // 2         # tree-half granularity for the product chain
INTERNAL = 255                 # internal nodes per tree
TOTAL = 511                    # all nodes per tree
DEPTH = 9
NODES = ENSEMBLE * INTERNAL    # 4080 decision columns
NCHUNK = 510                   # nodes per matmul (<=512 fp32 psum bank)
NCHUNKS = NODES // NCHUNK      # 8


def _sep_order():
    """Node order with each level's left children stored contiguously
    before its right children: makes every tree-product DVE op stride-1.
    Returns (sep, pos): sep[k] = heap index at separated position k,
    pos[j] = separated position of heap node j."""
    sep = [0]
    for lvl in range(1, DEPTH):
        prev = sep[(1 << (lvl - 1)) - 1:(1 << lvl) - 1]
        sep += [2 * p + 1 for p in prev] + [2 * p + 2 for p in prev]
    pos = np.empty(TOTAL, np.int64)
    for k, j in enumerate(sep):
        pos[j] = k
    return np.asarray(sep), pos


SEP, SEP_POS = _sep_order()

# dec columns live in level-major order across trees: (lvl, t, pos). The
# product chain's level op then depends only on its own sigmoid chunk(s).
# DEC_L[lvl] = column offset of level lvl; chunks are level-aligned.
DEC_L = [ENSEMBLE * ((1 << lvl) - 1) for lvl in range(DEPTH - 1)]
DEC_CHUNKS = [(0, 496), (496, 512), (1008, 512), (1520, 512),
              (2032, 512), (2544, 512), (3056, 512), (3568, 512)]


def _dec_perm():
    """(tree, heap-node) index arrays giving the level-major dec order."""
    t_idx, h_idx = [], []
    for lvl in range(DEPTH - 1):
        start = (1 << lvl) - 1
        for t in range(ENSEMBLE):
            for c in range(1 << lvl):
                t_idx.append(t)
                h_idx.append(SEP[start + c])
    return np.asarray(t_idx), np.asarray(h_idx)


DEC_T_IDX, DEC_H_IDX = _dec_perm()


def _split_bf16(a):
    """Exact split a = hi + lo with hi, lo bf16 (lo catches the tail)."""
    hi = a.astype(ml_dtypes.bfloat16)
    lo = (a - hi.astype(np.float32)).astype(ml_dtypes.bfloat16)
    return np.ascontiguousarray(hi), np.ascontiguousarray(lo)


def build_bass(with_nodeb, with_b2):
    nc = bacc.Bacc("TRN2", target_bir_lowering=False, debug=False,
                   num_devices=N_CORES)

    # K-padded, [128, NK, ...] layouts so each tensor loads in ONE dma
    xth_d = nc.dram_tensor("xth", [NK, 128, BC], BF16, kind="ExternalInput")
    xtl_d = nc.dram_tensor("xtl", [NK, 128, BC], BF16, kind="ExternalInput")
    w1h_d = nc.dram_tensor("w1h", [NK, 128, HID], BF16, kind="ExternalInput")
    w1l_d = nc.dram_tensor("w1l", [NK, 128, HID], BF16, kind="ExternalInput")
    b1_d = nc.dram_tensor("b1", [HID, 1], F32, kind="ExternalInput")
    w2_d = nc.dram_tensor("w2", [HID, CLS], F32, kind="ExternalInput")
    nwh_d = nc.dram_tensor("nwh", [HID, NODES], BF16, kind="ExternalInput")
    nwl_d = nc.dram_tensor("nwl", [HID, NODES], BF16, kind="ExternalInput")
    b2_d = (nc.dram_tensor("b2", [1, CLS], F32, kind="ExternalInput")
            if with_b2 else None)
    nb_d = (nc.dram_tensor("nb", [1, NODES], F32, kind="ExternalInput")
            if with_nodeb else None)

    pred_d = nc.dram_tensor("pred", [BC, CLS], F32, kind="ExternalOutput")
    res_d = nc.dram_tensor("res", [NB, 128, ENSEMBLE, TOTAL], F32,
                           kind="ExternalOutput")

    with tile.TileContext(nc) as tc, ExitStack() as ctx:
        consts = ctx.enter_context(tc.tile_pool(name="consts", bufs=1))
        xtp = ctx.enter_context(tc.tile_pool(name="xtp", bufs=1))
        fp = ctx.enter_context(tc.tile_pool(name="fp", bufs=1))
        decp = ctx.enter_context(tc.tile_pool(name="decp", bufs=3))
        resp = ctx.enter_context(tc.tile_pool(name="resp", bufs=2))
        outp = ctx.enter_context(tc.tile_pool(name="outp", bufs=2))

        # ---- PE warm-up/filler tiles (zeroed; outputs never read). The
        # HAM clock-gate halves the PE clock after ~3.4us idle; dummy
        # matmuls bridge the gaps so real matmuls run at 2.4 GHz. ----
        warm_ps = ctx.enter_context(
            tc.tile_pool(name="warm_ps", bufs=1, space="PSUM"))
        wlhs = consts.tile([128, 128], BF16, tag="wlhs")
        wrhs = consts.tile([128, 512], BF16, tag="wrhs")
        nc.vector.memset(wlhs, 0.0)
        nc.vector.memset(wrhs, 0.0)
        wps = warm_ps.tile([128, 512], F32, tag="warm")

        def pe_filler(n, after=None):
            first = last = None
            for j in range(n):
                mm = nc.tensor.matmul(wps, wlhs, wrhs,
                                      start=(j == 0), stop=(j == n - 1))
                if first is None:
                    first = mm
                last = mm
            if after is not None:
                add_dep_helper(first.ins, after.ins, sync=False,
                               reason="pe filler after real matmuls")
            return last

        warm_tail = pe_filler(28)

        # ---- input loads: one large DMA per tensor, on ScalarE's ring ----
        # DRAM [NK, 128, F] -> SBUF [128, NK, F] (partition = middle dim)
        w1h_sb = consts.tile([128, NK, HID], BF16, tag="w1h")
        w1l_sb = consts.tile([128, NK, HID], BF16, tag="w1l")
        xth_sb = xtp.tile([128, NK, BC], BF16, tag="xth")
        xtl_sb = xtp.tile([128, NK, BC], BF16, tag="xtl")
        nc.scalar.dma_start(out=xth_sb, in_=xth_d.rearrange("k p b -> p k b"))
        nc.scalar.dma_start(out=w1h_sb, in_=w1h_d.rearrange("k p h -> p k h"))
        nc.scalar.dma_start(out=w1l_sb, in_=w1l_d.rearrange("k p h -> p k h"))
        nc.scalar.dma_start(out=xtl_sb, in_=xtl_d.rearrange("k p b -> p k b"))
        nwh_sb = consts.tile([HID, NODES], BF16, tag="nwh")
        nwl_sb = consts.tile([HID, NODES], BF16, tag="nwl")
        nc.scalar.dma_start(out=nwh_sb, in_=nwh_d[:])
        nc.scalar.dma_start(out=nwl_sb, in_=nwl_d[:])
        b1_sb = consts.tile([HID, 1], F32, tag="b1")
        nc.scalar.dma_start(out=b1_sb, in_=b1_d[:])
        w2_sb = consts.tile([HID, CLS], F32, tag="w2")
        nc.scalar.dma_start(out=w2_sb, in_=w2_d[:])
        ones_sb = consts.tile([1, 128], F32, tag="ones")
        nc.vector.memset(ones_sb, 1.0)
        if with_b2:
            b2_sb = consts.tile([1, CLS], F32, tag="b2")
            nc.scalar.dma_start(out=b2_sb, in_=b2_d[:])
        if with_nodeb:
            nb_sb = consts.tile([1, NODES], F32, tag="nb")
            nc.scalar.dma_start(out=nb_sb, in_=nb_d[:])

        # ---- encoder + per-half feats hi/lo split ----
        featsT = fp.tile([HID, BC], F32, tag="featsT")
        fhi = fp.tile([HID, BC], BF16, tag="fhi")
        flo = fp.tile([HID, BC], BF16, tag="flo")
        with tc.tile_pool(name="enc_ps", bufs=2, space="PSUM") as enc_ps:
            for n in range(2):
                ps = enc_ps.tile([128, 512], F32, tag="enc")
                nsl = slice(n * 512, (n + 1) * 512)
                # xth-only terms first: they only need the first two loads
                mms = [(w1h_sb, xth_sb, k) for k in range(NK)]
                mms += [(w1l_sb, xth_sb, k) for k in range(NK)]
                mms += [(w1h_sb, xtl_sb, k) for k in range(NK)]
                for j, (wt, xt, k) in enumerate(mms):
                    nc.tensor.matmul(ps, wt[:, k, :], xt[:, k, nsl],
                                     start=(j == 0), stop=(j == len(mms) - 1))
                nc.scalar.activation(out=featsT[:, nsl], in_=ps,
                                     func=AF.Relu, bias=b1_sb, scale=1.0)
                fdiff = fp.tile([HID, 512], F32, tag=f"fdiff_{n}")
                nc.vector.tensor_copy(fhi[:, nsl], featsT[:, nsl])
                nc.vector.tensor_sub(fdiff, featsT[:, nsl], fhi[:, nsl])
                nc.vector.tensor_copy(flo[:, nsl], fdiff)

        # ---- aux prediction (tiny): pred[b, c] = feats @ W2 + b2 ----
        with tc.tile_pool(name="pred_ps", bufs=2, space="PSUM") as pred_ps:
            for i in range(NB):
                bsl = featsT[:, i * 128:(i + 1) * 128]
                pp = pred_ps.tile([128, CLS], F32, tag="pred")
                nc.tensor.matmul(pp, bsl, w2_sb, start=True, stop=not with_b2)
                if with_b2:
                    nc.tensor.matmul(pp, ones_sb, b2_sb, start=False,
                                     stop=True)
                po = outp.tile([128, CLS], F32, tag="po")
                nc.vector.tensor_copy(po, pp)
                nc.sync.dma_start(out=pred_d[i * 128:(i + 1) * 128, :], in_=po)

        # ---- trees, per 128-row batch tile ----
        # dec columns are level-major across trees; results live in
        # separated order per tree (host maps back to heap order).
        with tc.tile_pool(name="tree_ps", bufs=3, space="PSUM") as tree_ps:
            for i in range(NB):
                isl = slice(i * 128, (i + 1) * 128)
                dec = decp.tile([128, NODES], F32, tag="dec")
                last_mm = None
                for half in range(NCHUNKS // 2):
                    ps = tree_ps.tile([128, 2, 512], F32, tag="tree")
                    for c2 in range(2):
                        off, sz = DEC_CHUNKS[half * 2 + c2]
                        csl = slice(off, off + sz)
                        out_ps = ps[:, c2, :sz]
                        nc.tensor.matmul(out_ps, fhi[:, isl], nwh_sb[:, csl],
                                         start=True, stop=False)
                        nc.tensor.matmul(out_ps, flo[:, isl], nwh_sb[:, csl],
                                         start=False, stop=False)
                        last_mm = nc.tensor.matmul(
                            out_ps, fhi[:, isl], nwl_sb[:, csl],
                            start=False, stop=not with_nodeb)
                        if with_nodeb:
                            last_mm = nc.tensor.matmul(
                                out_ps, ones_sb, nb_sb[:, csl],
                                start=False, stop=True)
                        nc.scalar.activation(out=dec[:, csl], in_=out_ps,
                                             func=AF.Sigmoid)
                pe_filler(8, after=last_mm)

                res_sb = resp.tile([128, ENSEMBLE, TOTAL], F32, tag="res")
                nc.vector.memset(res_sb[:, :, 0:1], 1.0)
                for lvl in range(DEPTH - 1):
                    start = (1 << lvl) - 1
                    cnt = 1 << lvl
                    off = 2 * start + 1
                    parent = res_sb[:, :, start:start + cnt]
                    decs = dec[:, DEC_L[lvl]:DEC_L[lvl] + ENSEMBLE * cnt]
                    decs = decs.rearrange("p (t c) -> p t c", t=ENSEMBLE)
                    lefts = res_sb[:, :, off:off + cnt]
                    rights = res_sb[:, :, off + cnt:off + 2 * cnt]
                    nc.vector.tensor_mul(lefts, parent, decs)
                    if lvl == DEPTH - 2:
                        # last (biggest) level's subtract runs on the
                        # otherwise-idle GpSimd engine
                        nc.gpsimd.tensor_sub(rights, parent, lefts)
                    else:
                        nc.vector.tensor_sub(rights, parent, lefts)
                nc.sync.dma_start(out=res_d[i], in_=res_sb)

    nc.compile()
    return nc


_NC_CACHE = {}


def _get_nc(with_nodeb, with_b2):
    key = (with_nodeb, with_b2)
    if key not in _NC_CACHE:
        _NC_CACHE[key] = build_bass(with_nodeb, with_b2)
    return _NC_CACHE[key]


def _pad_k(a):
    """Zero-pad axis 0 from IN_DIM to KPAD and reshape to [NK, 128, ...]."""
    out = np.zeros((KPAD,) + a.shape[1:], a.dtype)
    out[:IN_DIM] = a
    return out.reshape((NK, 128) + a.shape[1:])


def run_on_device(inputs, trace=False):
    """Shard inputs, run the Bass kernel on 8 cores, gather full outputs.

    Returns (prediction, all_results, BassKernelResults)."""
    X = np.ascontiguousarray(np.asarray(inputs["X"], dtype=np.float32))
    W1 = np.ascontiguousarray(np.asarray(inputs["W1"], dtype=np.float32))
    b1 = np.ascontiguousarray(
        np.asarray(inputs["b1"], dtype=np.float32).reshape(HID, 1))
    W2 = np.ascontiguousarray(np.asarray(inputs["W2"], dtype=np.float32))
    b2 = np.ascontiguousarray(
        np.asarray(inputs["b2"], dtype=np.float32).reshape(1, CLS))
    # nodeW/nodeb columns in level-major dec order (see _dec_perm)
    nodeW = np.asarray(inputs["nodeW"],
                       dtype=np.float32)[DEC_T_IDX, DEC_H_IDX, :]  # [4080,128]
    nodeb = np.ascontiguousarray(np.asarray(
        inputs["nodeb"], dtype=np.float32)[DEC_T_IDX,
                                           DEC_H_IDX].reshape(1, NODES))

    XT = np.ascontiguousarray(X.T)                           # [784, 8192]
    xth, xtl = _split_bf16(XT)
    w1h, w1l = _split_bf16(W1)
    xth, xtl = _pad_k(xth), _pad_k(xtl)                      # [7, 128, 8192]
    w1h, w1l = _pad_k(w1h), _pad_k(w1l)                      # [7, 128, 128]
    nwt = np.ascontiguousarray(nodeW.T)                      # [128, 4080]
    nwh, nwl = _split_bf16(nwt)

    with_nodeb = bool(np.any(nodeb))
    with_b2 = bool(np.any(b2))

    in_maps = []
    for c in range(N_CORES):
        m = {
            "xth": np.ascontiguousarray(xth[:, :, c * BC:(c + 1) * BC]),
            "xtl": np.ascontiguousarray(xtl[:, :, c * BC:(c + 1) * BC]),
            "w1h": w1h, "w1l": w1l, "b1": b1, "w2": W2,
            "nwh": nwh, "nwl": nwl,
        }
        if with_b2:
            m["b2"] = b2
        if with_nodeb:
            m["nb"] = nodeb
        in_maps.append(m)

    nc = _get_nc(with_nodeb, with_b2)
    r = run_bass_kernel_spmd(nc, in_maps, core_ids=list(range(N_CORES)),
                             trace=trace)

    prediction = np.empty((BATCH, CLS), np.float32)
    all_results = np.empty((ENSEMBLE, TOTAL, BATCH), np.float32)
    for c in range(N_CORES):
        prediction[c * BC:(c + 1) * BC] = r.results[c]["pred"]
        # res: [NB, 128, 16, 511] (sep node order) -> [16, 511, BC] (heap)
        rc = r.results[c]["res"].reshape(BC, ENSEMBLE, TOTAL)
        all_results[:, :, c * BC:(c + 1) * BC] = \
            rc.transpose(1, 2, 0)[:, SEP_POS, :]
    return prediction, all_results, r


def kernel(**inputs):
    prediction, all_results, _ = run_on_device(inputs, trace=False)
    return prediction, all_results


# revision 27
# speedup vs baseline: 1.1325x; 1.1325x over previous
"""Trainium2 Bass kernel for DeepLACForests (8-core data-parallel over batch).

Computation (matches the reference nn.Module):
  feats = relu(X @ W1 + b1)                  [B, 128]
  prediction = feats @ W2 + b2               [B, 10]
  dec = sigmoid(einsum('bd,tid->tib', feats, nodeW) + nodeb)   [16, 255, B]
  all_results = level-wise prefix product down a depth-9 heap  [16, 511, B]

Sharding: data-parallel over batch (1024 rows/core x 8 cores); encoder and
node weights (~2MB) replicated. On-device layout keeps batch on SBUF
partitions and (tree, node) on the free dimension so every DMA is fully
contiguous; the host pre-transposes X / nodeW and untransposes the result.

Matmuls run as bf16 hi/lo split-precision (x = hi + lo exactly, with
x @ w ~= hi@whi + lo@whi + hi@wlo, ~2^-17 relative error): fp32 matmuls
stream at half rate on the PE, so three bf16 passes beat one fp32 pass
while keeping near-fp32 accuracy.

DMA discipline: inputs arrive in a handful of large multi-dim DMAs issued
from ScalarE's HW-DGE ring; result tiles stream out per tree-half (16KB
contiguous per partition) from SyncE's ring, overlapping the product chain.
"""

from contextlib import ExitStack

import ml_dtypes
import numpy as np

import concourse.bass as bass
import concourse.tile as tile
from concourse import bacc, mybir
from concourse.bass_utils import run_bass_kernel_spmd
from concourse.tile_rust import add_dep_helper

F32 = mybir.dt.float32
BF16 = mybir.dt.bfloat16
AF = mybir.ActivationFunctionType

N_CORES = 8
BATCH = 8192
BC = BATCH // N_CORES          # 1024 batch rows per core
NB = BC // 128                 # 8 partition tiles per core
IN_DIM = 784
KPAD = 896                     # 7 * 128: zero-padded contraction dim
NK = KPAD // 128               # 7 uniform K chunks
HID = 128
CLS = 10
ENSEMBLE = 16
HALF_T = ENSEMBLE // 2         # tree-half granularity for the product chain
INTERNAL = 255                 # internal nodes per tree
TOTAL = 511                    # all nodes per tree
DEPTH = 9
NODES = ENSEMBLE * INTERNAL    # 4080 decision columns
NCHUNK = 510                   # nodes per matmul (<=512 fp32 psum bank)
NCHUNKS = NODES // NCHUNK      # 8


def _sep_order():
    """Node order with each level's left children stored contiguously
    before its right children: makes every tree-product DVE op stride-1.
    Returns (sep, pos): sep[k] = heap index at separated position k,
    pos[j] = separated position of heap node j."""
    sep = [0]
    for lvl in range(1, DEPTH):
        prev = sep[(1 << (lvl - 1)) - 1:(1 << lvl) - 1]
        sep += [2 * p + 1 for p in prev] + [2 * p + 2 for p in prev]
    pos = np.empty(TOTAL, np.int64)
    for k, j in enumerate(sep):
        pos[j] = k
    return np.asarray(sep), pos


SEP, SEP_POS = _sep_order()

# dec columns live in level-major order across trees: (lvl, t, pos). The
# product chain's level op then depends only on its own sigmoid chunk(s).
# DEC_L[lvl] = column offset of level lvl; chunks are level-aligned.
DEC_L = [ENSEMBLE * ((1 << lvl) - 1) for lvl in range(DEPTH - 1)]
DEC_CHUNKS = [(0, 496), (496, 512), (1008, 512), (1520, 512),
              (2032, 512), (2544, 512), (3056, 512), (3568, 512)]


def _dec_perm():
    """(tree, heap-node) index arrays giving the level-major, tree-minor
    dec order (matching the [node-position, tree] result layout)."""
    t_idx, h_idx = [], []
    for lvl in range(DEPTH - 1):
        start = (1 << lvl) - 1
        for c in range(1 << lvl):
            for t in range(ENSEMBLE):
                t_idx.append(t)
                h_idx.append(SEP[start + c])
    return np.asarray(t_idx), np.asarray(h_idx)


DEC_T_IDX, DEC_H_IDX = _dec_perm()
LEAF_POS = TOTAL - (1 << (DEPTH - 1))   # 255: first leaf-level position


def _split_bf16(a):
    """Exact split a = hi + lo with hi, lo bf16 (lo catches the tail)."""
    hi = a.astype(ml_dtypes.bfloat16)
    lo = (a - hi.astype(np.float32)).astype(ml_dtypes.bfloat16)
    return np.ascontiguousarray(hi), np.ascontiguousarray(lo)


def build_bass(with_nodeb, with_b2):
    nc = bacc.Bacc("TRN2", target_bir_lowering=False, debug=False,
                   num_devices=N_CORES)

    # K-padded, [128, NK, ...] layouts so each tensor loads in ONE dma
    xth_d = nc.dram_tensor("xth", [NK, 128, BC], BF16, kind="ExternalInput")
    xtl_d = nc.dram_tensor("xtl", [NK, 128, BC], BF16, kind="ExternalInput")
    w1h_d = nc.dram_tensor("w1h", [NK, 128, HID], BF16, kind="ExternalInput")
    w1l_d = nc.dram_tensor("w1l", [NK, 128, HID], BF16, kind="ExternalInput")
    b1_d = nc.dram_tensor("b1", [HID, 1], F32, kind="ExternalInput")
    w2_d = nc.dram_tensor("w2", [HID, CLS], F32, kind="ExternalInput")
    nwh_d = nc.dram_tensor("nwh", [HID, NODES], BF16, kind="ExternalInput")
    nwl_d = nc.dram_tensor("nwl", [HID, NODES], BF16, kind="ExternalInput")
    b2_d = (nc.dram_tensor("b2", [1, CLS], F32, kind="ExternalInput")
            if with_b2 else None)
    nb_d = (nc.dram_tensor("nb", [1, NODES], F32, kind="ExternalInput")
            if with_nodeb else None)

    pred_d = nc.dram_tensor("pred", [BC, CLS], F32, kind="ExternalOutput")
    res_d = nc.dram_tensor("res", [NB, 128, TOTAL, ENSEMBLE], F32,
                           kind="ExternalOutput")

    with tile.TileContext(nc) as tc, ExitStack() as ctx:
        consts = ctx.enter_context(tc.tile_pool(name="consts", bufs=1))
        xtp = ctx.enter_context(tc.tile_pool(name="xtp", bufs=1))
        fp = ctx.enter_context(tc.tile_pool(name="fp", bufs=1))
        decp = ctx.enter_context(tc.tile_pool(name="decp", bufs=3))
        resp = ctx.enter_context(tc.tile_pool(name="resp", bufs=2))
        outp = ctx.enter_context(tc.tile_pool(name="outp", bufs=2))

        # ---- input loads: one large DMA per tensor, on ScalarE's ring ----
        # DRAM [NK, 128, F] -> SBUF [128, NK, F] (partition = middle dim)
        w1h_sb = consts.tile([128, NK, HID], BF16, tag="w1h")
        w1l_sb = consts.tile([128, NK, HID], BF16, tag="w1l")
        xth_sb = xtp.tile([128, NK, BC], BF16, tag="xth")
        xtl_sb = xtp.tile([128, NK, BC], BF16, tag="xtl")
        nc.scalar.dma_start(out=xth_sb, in_=xth_d.rearrange("k p b -> p k b"))
        nc.scalar.dma_start(out=w1h_sb, in_=w1h_d.rearrange("k p h -> p k h"))
        nc.scalar.dma_start(out=w1l_sb, in_=w1l_d.rearrange("k p h -> p k h"))
        nc.scalar.dma_start(out=xtl_sb, in_=xtl_d.rearrange("k p b -> p k b"))
        nwh_sb = consts.tile([HID, NODES], BF16, tag="nwh")
        nwl_sb = consts.tile([HID, NODES], BF16, tag="nwl")
        nc.scalar.dma_start(out=nwh_sb, in_=nwh_d[:])
        nc.scalar.dma_start(out=nwl_sb, in_=nwl_d[:])
        b1_sb = consts.tile([HID, 1], F32, tag="b1")
        nc.scalar.dma_start(out=b1_sb, in_=b1_d[:])
        w2_sb = consts.tile([HID, CLS], F32, tag="w2")
        nc.scalar.dma_start(out=w2_sb, in_=w2_d[:])
        ones_sb = consts.tile([1, 128], F32, tag="ones")
        nc.vector.memset(ones_sb, 1.0)
        if with_b2:
            b2_sb = consts.tile([1, CLS], F32, tag="b2")
            nc.scalar.dma_start(out=b2_sb, in_=b2_d[:])
        if with_nodeb:
            nb_sb = consts.tile([1, NODES], F32, tag="nb")
            nc.scalar.dma_start(out=nb_sb, in_=nb_d[:])

        # ---- encoder + per-half feats hi/lo split ----
        featsT = fp.tile([HID, BC], F32, tag="featsT")
        fhi = fp.tile([HID, BC], BF16, tag="fhi")
        flo = fp.tile([HID, BC], BF16, tag="flo")
        with tc.tile_pool(name="enc_ps", bufs=2, space="PSUM") as enc_ps:
            for n in range(2):
                ps = enc_ps.tile([128, 512], F32, tag="enc")
                nsl = slice(n * 512, (n + 1) * 512)
                # xth-only terms first: they only need the first two loads
                mms = [(w1h_sb, xth_sb, k) for k in range(NK)]
                mms += [(w1l_sb, xth_sb, k) for k in range(NK)]
                mms += [(w1h_sb, xtl_sb, k) for k in range(NK)]
                for j, (wt, xt, k) in enumerate(mms):
                    nc.tensor.matmul(ps, wt[:, k, :], xt[:, k, nsl],
                                     start=(j == 0), stop=(j == len(mms) - 1))
                nc.scalar.activation(out=featsT[:, nsl], in_=ps,
                                     func=AF.Relu, bias=b1_sb, scale=1.0)
                fdiff = fp.tile([HID, 512], F32, tag=f"fdiff_{n}")
                nc.vector.tensor_copy(fhi[:, nsl], featsT[:, nsl])
                nc.vector.tensor_sub(fdiff, featsT[:, nsl], fhi[:, nsl])
                nc.vector.tensor_copy(flo[:, nsl], fdiff)

        # ---- aux prediction (tiny): pred[b, c] = feats @ W2 + b2 ----
        with tc.tile_pool(name="pred_ps", bufs=2, space="PSUM") as pred_ps:
            for i in range(NB):
                bsl = featsT[:, i * 128:(i + 1) * 128]
                pp = pred_ps.tile([128, CLS], F32, tag="pred")
                nc.tensor.matmul(pp, bsl, w2_sb, start=True, stop=not with_b2)
                if with_b2:
                    nc.tensor.matmul(pp, ones_sb, b2_sb, start=False,
                                     stop=True)
                po = outp.tile([128, CLS], F32, tag="po")
                nc.vector.tensor_copy(po, pp)
                nc.sync.dma_start(out=pred_d[i * 128:(i + 1) * 128, :], in_=po)

        # ---- trees, per 128-row batch tile ----
        # dec columns are level-major across trees; results live in
        # separated order per tree (host maps back to heap order).
        with tc.tile_pool(name="tree_ps", bufs=3, space="PSUM") as tree_ps:
            for i in range(NB):
                isl = slice(i * 128, (i + 1) * 128)
                dec = decp.tile([128, NODES], F32, tag="dec")
                for half in range(NCHUNKS // 2):
                    ps = tree_ps.tile([128, 2, 512], F32, tag="tree")
                    for c2 in range(2):
                        off, sz = DEC_CHUNKS[half * 2 + c2]
                        csl = slice(off, off + sz)
                        out_ps = ps[:, c2, :sz]
                        nc.tensor.matmul(out_ps, fhi[:, isl], nwh_sb[:, csl],
                                         start=True, stop=False)
                        nc.tensor.matmul(out_ps, flo[:, isl], nwh_sb[:, csl],
                                         start=False, stop=False)
                        nc.tensor.matmul(out_ps, fhi[:, isl], nwl_sb[:, csl],
                                         start=False, stop=not with_nodeb)
                        if with_nodeb:
                            nc.tensor.matmul(out_ps, ones_sb, nb_sb[:, csl],
                                             start=False, stop=True)
                        nc.scalar.activation(out=dec[:, csl], in_=out_ps,
                                             func=AF.Sigmoid)

                # results in [node-position, tree] layout: every level op is
                # one flat contiguous DVE op, and the output leaves in two
                # contiguous pieces (internal levels early, leaves at end)
                res_sb = resp.tile([128, TOTAL, ENSEMBLE], F32, tag="res")
                nc.vector.memset(res_sb[:, 0, :], 1.0)
                for lvl in range(DEPTH - 1):
                    start = (1 << lvl) - 1
                    cnt = 1 << lvl
                    off = 2 * start + 1
                    w = cnt * ENSEMBLE
                    flat = res_sb.rearrange("p n t -> p (n t)")
                    parent = flat[:, start * ENSEMBLE:start * ENSEMBLE + w]
                    decs = dec[:, DEC_L[lvl]:DEC_L[lvl] + w]
                    lefts = flat[:, off * ENSEMBLE:off * ENSEMBLE + w]
                    rights = flat[:, (off + cnt) * ENSEMBLE:
                                  (off + cnt) * ENSEMBLE + w]
                    nc.vector.tensor_mul(lefts, parent, decs)
                    nc.vector.tensor_sub(rights, parent, lefts)
                    if lvl == DEPTH - 3:
                        # levels 0..7 (positions 0..254) are final: start
                        # streaming them while the leaf level computes
                        nc.sync.dma_start(out=res_d[i][:, :LEAF_POS, :],
                                          in_=res_sb[:, :LEAF_POS, :])
                nc.sync.dma_start(out=res_d[i][:, LEAF_POS:, :],
                                  in_=res_sb[:, LEAF_POS:, :])

    nc.compile()
    return nc


_NC_CACHE = {}


def _get_nc(with_nodeb, with_b2):
    key = (with_nodeb, with_b2)
    if key not in _NC_CACHE:
        _NC_CACHE[key] = build_bass(with_nodeb, with_b2)
    return _NC_CACHE[key]


def _pad_k(a):
    """Zero-pad axis 0 from IN_DIM to KPAD and reshape to [NK, 128, ...]."""
    out = np.zeros((KPAD,) + a.shape[1:], a.dtype)
    out[:IN_DIM] = a
    return out.reshape((NK, 128) + a.shape[1:])


def run_on_device(inputs, trace=False):
    """Shard inputs, run the Bass kernel on 8 cores, gather full outputs.

    Returns (prediction, all_results, BassKernelResults)."""
    X = np.ascontiguousarray(np.asarray(inputs["X"], dtype=np.float32))
    W1 = np.ascontiguousarray(np.asarray(inputs["W1"], dtype=np.float32))
    b1 = np.ascontiguousarray(
        np.asarray(inputs["b1"], dtype=np.float32).reshape(HID, 1))
    W2 = np.ascontiguousarray(np.asarray(inputs["W2"], dtype=np.float32))
    b2 = np.ascontiguousarray(
        np.asarray(inputs["b2"], dtype=np.float32).reshape(1, CLS))
    # nodeW/nodeb columns in level-major dec order (see _dec_perm)
    nodeW = np.asarray(inputs["nodeW"],
                       dtype=np.float32)[DEC_T_IDX, DEC_H_IDX, :]  # [4080,128]
    nodeb = np.ascontiguousarray(np.asarray(
        inputs["nodeb"], dtype=np.float32)[DEC_T_IDX,
                                           DEC_H_IDX].reshape(1, NODES))

    XT = np.ascontiguousarray(X.T)                           # [784, 8192]
    xth, xtl = _split_bf16(XT)
    w1h, w1l = _split_bf16(W1)
    xth, xtl = _pad_k(xth), _pad_k(xtl)                      # [7, 128, 8192]
    w1h, w1l = _pad_k(w1h), _pad_k(w1l)                      # [7, 128, 128]
    nwt = np.ascontiguousarray(nodeW.T)                      # [128, 4080]
    nwh, nwl = _split_bf16(nwt)

    with_nodeb = bool(np.any(nodeb))
    with_b2 = bool(np.any(b2))

    in_maps = []
    for c in range(N_CORES):
        m = {
            "xth": np.ascontiguousarray(xth[:, :, c * BC:(c + 1) * BC]),
            "xtl": np.ascontiguousarray(xtl[:, :, c * BC:(c + 1) * BC]),
            "w1h": w1h, "w1l": w1l, "b1": b1, "w2": W2,
            "nwh": nwh, "nwl": nwl,
        }
        if with_b2:
            m["b2"] = b2
        if with_nodeb:
            m["nb"] = nodeb
        in_maps.append(m)

    nc = _get_nc(with_nodeb, with_b2)
    r = run_bass_kernel_spmd(nc, in_maps, core_ids=list(range(N_CORES)),
                             trace=trace)

    prediction = np.empty((BATCH, CLS), np.float32)
    all_results = np.empty((ENSEMBLE, TOTAL, BATCH), np.float32)
    for c in range(N_CORES):
        prediction[c * BC:(c + 1) * BC] = r.results[c]["pred"]
        # res: [NB, 128, 511, 16] (sep node positions) -> [16, 511, BC]
        rc = r.results[c]["res"].reshape(BC, TOTAL, ENSEMBLE)
        all_results[:, :, c * BC:(c + 1) * BC] = \
            rc.transpose(2, 1, 0)[:, SEP_POS, :]
    return prediction, all_results, r


def kernel(**inputs):
    prediction, all_results, _ = run_on_device(inputs, trace=False)
    return prediction, all_results


# revision 30
# speedup vs baseline: 1.1736x; 1.0363x over previous
"""Trainium2 Bass kernel for DeepLACForests (8-core data-parallel over batch).

Computation (matches the reference nn.Module):
  feats = relu(X @ W1 + b1)                  [B, 128]
  prediction = feats @ W2 + b2               [B, 10]
  dec = sigmoid(einsum('bd,tid->tib', feats, nodeW) + nodeb)   [16, 255, B]
  all_results = level-wise prefix product down a depth-9 heap  [16, 511, B]

Sharding: data-parallel over batch (1024 rows/core x 8 cores); encoder and
node weights (~2MB) replicated. On-device layout keeps batch on SBUF
partitions and (tree, node) on the free dimension so every DMA is fully
contiguous; the host pre-transposes X / nodeW and untransposes the result.

Matmuls run as bf16 hi/lo split-precision (x = hi + lo exactly, with
x @ w ~= hi@whi + lo@whi + hi@wlo, ~2^-17 relative error): fp32 matmuls
stream at half rate on the PE, so three bf16 passes beat one fp32 pass
while keeping near-fp32 accuracy.

DMA discipline: inputs arrive in a handful of large multi-dim DMAs issued
from ScalarE's HW-DGE ring; result tiles stream out per tree-half (16KB
contiguous per partition) from SyncE's ring, overlapping the product chain.
"""

from contextlib import ExitStack

import ml_dtypes
import numpy as np

import concourse.bass as bass
import concourse.tile as tile
from concourse import bacc, mybir
from concourse.bass_utils import run_bass_kernel_spmd
from concourse.tile_rust import add_dep_helper

F32 = mybir.dt.float32
BF16 = mybir.dt.bfloat16
AF = mybir.ActivationFunctionType

N_CORES = 8
BATCH = 8192
BC = BATCH // N_CORES          # 1024 batch rows per core
NB = BC // 128                 # 8 partition tiles per core
IN_DIM = 784
KPAD = 896                     # 7 * 128: zero-padded contraction dim
NK = KPAD // 128               # 7 uniform K chunks
HID = 128
CLS = 10
ENSEMBLE = 16
HALF_T = ENSEMBLE // 2         # tree-half granularity for the product chain
INTERNAL = 255                 # internal nodes per tree
TOTAL = 511                    # all nodes per tree
DEPTH = 9
NODES = ENSEMBLE * INTERNAL    # 4080 decision columns
NCHUNK = 510                   # nodes per matmul (<=512 fp32 psum bank)
NCHUNKS = NODES // NCHUNK      # 8


def _sep_order():
    """Node order with each level's left children stored contiguously
    before its right children: makes every tree-product DVE op stride-1.
    Returns (sep, pos): sep[k] = heap index at separated position k,
    pos[j] = separated position of heap node j."""
    sep = [0]
    for lvl in range(1, DEPTH):
        prev = sep[(1 << (lvl - 1)) - 1:(1 << lvl) - 1]
        sep += [2 * p + 1 for p in prev] + [2 * p + 2 for p in prev]
    pos = np.empty(TOTAL, np.int64)
    for k, j in enumerate(sep):
        pos[j] = k
    return np.asarray(sep), pos


SEP, SEP_POS = _sep_order()

# dec columns live in level-major order across trees: (lvl, t, pos). The
# product chain's level op then depends only on its own sigmoid chunk(s).
# DEC_L[lvl] = column offset of level lvl; chunks are level-aligned.
DEC_L = [ENSEMBLE * ((1 << lvl) - 1) for lvl in range(DEPTH - 1)]
DEC_CHUNKS = [(0, 496), (496, 512), (1008, 512), (1520, 512),
              (2032, 512), (2544, 512), (3056, 512), (3568, 512)]


def _dec_perm():
    """(tree, heap-node) index arrays giving the level-major, tree-minor
    dec order (matching the [node-position, tree] result layout)."""
    t_idx, h_idx = [], []
    for lvl in range(DEPTH - 1):
        start = (1 << lvl) - 1
        for c in range(1 << lvl):
            for t in range(ENSEMBLE):
                t_idx.append(t)
                h_idx.append(SEP[start + c])
    return np.asarray(t_idx), np.asarray(h_idx)


DEC_T_IDX, DEC_H_IDX = _dec_perm()
LEAF_POS = TOTAL - (1 << (DEPTH - 1))   # 255: first leaf-level position


def _split_bf16(a):
    """Exact split a = hi + lo with hi, lo bf16 (lo catches the tail)."""
    hi = a.astype(ml_dtypes.bfloat16)
    lo = (a - hi.astype(np.float32)).astype(ml_dtypes.bfloat16)
    return np.ascontiguousarray(hi), np.ascontiguousarray(lo)


def build_bass(with_nodeb, with_b2):
    nc = bacc.Bacc("TRN2", target_bir_lowering=False, debug=False,
                   num_devices=N_CORES)

    # K-padded, [128, NK, ...] layouts so each tensor loads in ONE dma
    xth_d = nc.dram_tensor("xth", [NK, 128, BC], BF16, kind="ExternalInput")
    xtl_d = nc.dram_tensor("xtl", [NK, 128, BC], BF16, kind="ExternalInput")
    w1h_d = nc.dram_tensor("w1h", [NK, 128, HID], BF16, kind="ExternalInput")
    w1l_d = nc.dram_tensor("w1l", [NK, 128, HID], BF16, kind="ExternalInput")
    b1_d = nc.dram_tensor("b1", [HID, 1], F32, kind="ExternalInput")
    w2_d = nc.dram_tensor("w2", [HID, CLS], F32, kind="ExternalInput")
    nwh_d = nc.dram_tensor("nwh", [HID, NODES], BF16, kind="ExternalInput")
    nwl_d = nc.dram_tensor("nwl", [HID, NODES], BF16, kind="ExternalInput")
    b2_d = (nc.dram_tensor("b2", [1, CLS], F32, kind="ExternalInput")
            if with_b2 else None)
    nb_d = (nc.dram_tensor("nb", [1, NODES], F32, kind="ExternalInput")
            if with_nodeb else None)

    pred_d = nc.dram_tensor("pred", [BC, CLS], F32, kind="ExternalOutput")
    res_d = nc.dram_tensor("res", [NB, 128, TOTAL, ENSEMBLE], F32,
                           kind="ExternalOutput")

    with tile.TileContext(nc) as tc, ExitStack() as ctx:
        consts = ctx.enter_context(tc.tile_pool(name="consts", bufs=1))
        xtp = ctx.enter_context(tc.tile_pool(name="xtp", bufs=1))
        fp = ctx.enter_context(tc.tile_pool(name="fp", bufs=1))
        decp = ctx.enter_context(tc.tile_pool(name="decp", bufs=3))
        resp = ctx.enter_context(tc.tile_pool(name="resp", bufs=2))
        outp = ctx.enter_context(tc.tile_pool(name="outp", bufs=2))
        enc_ps = ctx.enter_context(
            tc.tile_pool(name="enc_ps", bufs=2, space="PSUM"))
        tree_ps = ctx.enter_context(
            tc.tile_pool(name="tree_ps", bufs=3, space="PSUM"))

        # ---- input loads: one large DMA per tensor, on ScalarE's ring ----
        # DRAM [NK, 128, F] -> SBUF [128, NK, F] (partition = middle dim)
        w1h_sb = consts.tile([128, NK, HID], BF16, tag="w1h")
        w1l_sb = consts.tile([128, NK, HID], BF16, tag="w1l")
        xth_sb = xtp.tile([128, NK, BC], BF16, tag="xth")
        xtl_sb = xtp.tile([128, NK, BC], BF16, tag="xtl")
        nc.scalar.dma_start(out=xth_sb, in_=xth_d.rearrange("k p b -> p k b"))
        nc.scalar.dma_start(out=w1h_sb, in_=w1h_d.rearrange("k p h -> p k h"))
        nc.scalar.dma_start(out=w1l_sb, in_=w1l_d.rearrange("k p h -> p k h"))
        nc.scalar.dma_start(out=xtl_sb, in_=xtl_d.rearrange("k p b -> p k b"))
        nwh_sb = consts.tile([HID, NODES], BF16, tag="nwh")
        nwl_sb = consts.tile([HID, NODES], BF16, tag="nwl")
        nc.scalar.dma_start(out=nwh_sb, in_=nwh_d[:])
        nc.scalar.dma_start(out=nwl_sb, in_=nwl_d[:])
        b1_sb = consts.tile([HID, 1], F32, tag="b1")
        nc.scalar.dma_start(out=b1_sb, in_=b1_d[:])
        w2_sb = consts.tile([HID, CLS], F32, tag="w2")
        nc.scalar.dma_start(out=w2_sb, in_=w2_d[:])
        ones_sb = consts.tile([1, 128], F32, tag="ones")
        nc.vector.memset(ones_sb, 1.0)
        if with_b2:
            b2_sb = consts.tile([1, CLS], F32, tag="b2")
            nc.scalar.dma_start(out=b2_sb, in_=b2_d[:])
        if with_nodeb:
            nb_sb = consts.tile([1, NODES], F32, tag="nb")
            nc.scalar.dma_start(out=nb_sb, in_=nb_d[:])

        # ---- encoder + per-half feats hi/lo split ----
        featsT = fp.tile([HID, BC], F32, tag="featsT")
        fhi = fp.tile([HID, BC], BF16, tag="fhi")
        flo = fp.tile([HID, BC], BF16, tag="flo")

        def encode_half(n):
            ps = enc_ps.tile([128, 512], F32, tag="enc")
            nsl = slice(n * 512, (n + 1) * 512)
            # xth-only terms first: they only need the first two loads
            mms = [(w1h_sb, xth_sb, k) for k in range(NK)]
            mms += [(w1l_sb, xth_sb, k) for k in range(NK)]
            mms += [(w1h_sb, xtl_sb, k) for k in range(NK)]
            for j, (wt, xt, k) in enumerate(mms):
                nc.tensor.matmul(ps, wt[:, k, :], xt[:, k, nsl],
                                 start=(j == 0), stop=(j == len(mms) - 1))
            nc.scalar.activation(out=featsT[:, nsl], in_=ps,
                                 func=AF.Relu, bias=b1_sb, scale=1.0)
            fdiff = fp.tile([HID, 512], F32, tag=f"fdiff_{n}")
            nc.vector.tensor_copy(fhi[:, nsl], featsT[:, nsl])
            nc.vector.tensor_sub(fdiff, featsT[:, nsl], fhi[:, nsl])
            nc.vector.tensor_copy(flo[:, nsl], fdiff)

        def pred_phase():
            # tiny: pred[b, c] = feats @ W2 + b2 (borrows a tree psum slot)
            for i in range(NB):
                bsl = featsT[:, i * 128:(i + 1) * 128]
                ppt = tree_ps.tile([128, 2, 512], F32, tag="tree")
                pp = ppt[:, 0, :CLS]
                nc.tensor.matmul(pp, bsl, w2_sb, start=True, stop=not with_b2)
                if with_b2:
                    nc.tensor.matmul(pp, ones_sb, b2_sb, start=False,
                                     stop=True)
                po = outp.tile([128, CLS], F32, tag="po")
                nc.vector.tensor_copy(po, pp)
                nc.sync.dma_start(out=pred_d[i * 128:(i + 1) * 128, :], in_=po)

        def tree_tile(i, split_tail=False):
            isl = slice(i * 128, (i + 1) * 128)
            dec = decp.tile([128, NODES], F32, tag="dec")
            for half in range(NCHUNKS // 2):
                ps = tree_ps.tile([128, 2, 512], F32, tag="tree")
                for c2 in range(2):
                    off, sz = DEC_CHUNKS[half * 2 + c2]
                    csl = slice(off, off + sz)
                    out_ps = ps[:, c2, :sz]
                    nc.tensor.matmul(out_ps, fhi[:, isl], nwh_sb[:, csl],
                                     start=True, stop=False)
                    nc.tensor.matmul(out_ps, flo[:, isl], nwh_sb[:, csl],
                                     start=False, stop=False)
                    nc.tensor.matmul(out_ps, fhi[:, isl], nwl_sb[:, csl],
                                     start=False, stop=not with_nodeb)
                    if with_nodeb:
                        nc.tensor.matmul(out_ps, ones_sb, nb_sb[:, csl],
                                         start=False, stop=True)
                    nc.scalar.activation(out=dec[:, csl], in_=out_ps,
                                         func=AF.Sigmoid)

            # results in [node-position, tree] layout: every level op is
            # one flat contiguous DVE op, and the output leaves in two
            # contiguous pieces (internal levels early, leaves at end)
            res_sb = resp.tile([128, TOTAL, ENSEMBLE], F32, tag="res")
            nc.vector.memset(res_sb[:, 0, :], 1.0)
            for lvl in range(DEPTH - 1):
                start = (1 << lvl) - 1
                cnt = 1 << lvl
                off = 2 * start + 1
                w = cnt * ENSEMBLE
                flat = res_sb.rearrange("p n t -> p (n t)")
                parent = flat[:, start * ENSEMBLE:start * ENSEMBLE + w]
                decs = dec[:, DEC_L[lvl]:DEC_L[lvl] + w]
                lefts = flat[:, off * ENSEMBLE:off * ENSEMBLE + w]
                rights = flat[:, (off + cnt) * ENSEMBLE:
                              (off + cnt) * ENSEMBLE + w]
                if lvl == DEPTH - 2 and split_tail:
                    # leaf lefts occupy [LEAF_POS, mid); rights [mid, TOTAL):
                    # stream lefts while the subtract runs
                    mid = off + cnt
                    nc.vector.tensor_mul(lefts, parent, decs)
                    nc.sync.dma_start(out=res_d[i][:, LEAF_POS:mid, :],
                                      in_=res_sb[:, LEAF_POS:mid, :])
                    nc.vector.tensor_sub(rights, parent, lefts)
                    nc.sync.dma_start(out=res_d[i][:, mid:, :],
                                      in_=res_sb[:, mid:, :])
                    return
                nc.vector.tensor_mul(lefts, parent, decs)
                nc.vector.tensor_sub(rights, parent, lefts)
                if lvl == DEPTH - 3:
                    # levels 0..7 (positions 0..254) are final: start
                    # streaming them while the leaf level computes
                    nc.sync.dma_start(out=res_d[i][:, :LEAF_POS, :],
                                      in_=res_sb[:, :LEAF_POS, :])
            nc.sync.dma_start(out=res_d[i][:, LEAF_POS:, :],
                              in_=res_sb[:, LEAF_POS:, :])

        # interleave so tree tile 0 starts as soon as feats half 0 exists
        encode_half(0)
        tree_tile(0)
        encode_half(1)
        pred_phase()
        for i in range(1, NB):
            tree_tile(i, split_tail=(i == NB - 1))

    nc.compile()
    return nc


_NC_CACHE = {}


def _get_nc(with_nodeb, with_b2):
    key = (with_nodeb, with_b2)
    if key not in _NC_CACHE:
        _NC_CACHE[key] = build_bass(with_nodeb, with_b2)
    return _NC_CACHE[key]


def _pad_k(a):
    """Zero-pad axis 0 from IN_DIM to KPAD and reshape to [NK, 128, ...]."""
    out = np.zeros((KPAD,) + a.shape[1:], a.dtype)
    out[:IN_DIM] = a
    return out.reshape((NK, 128) + a.shape[1:])


def run_on_device(inputs, trace=False):
    """Shard inputs, run the Bass kernel on 8 cores, gather full outputs.

    Returns (prediction, all_results, BassKernelResults)."""
    X = np.ascontiguousarray(np.asarray(inputs["X"], dtype=np.float32))
    W1 = np.ascontiguousarray(np.asarray(inputs["W1"], dtype=np.float32))
    b1 = np.ascontiguousarray(
        np.asarray(inputs["b1"], dtype=np.float32).reshape(HID, 1))
    W2 = np.ascontiguousarray(np.asarray(inputs["W2"], dtype=np.float32))
    b2 = np.ascontiguousarray(
        np.asarray(inputs["b2"], dtype=np.float32).reshape(1, CLS))
    # nodeW/nodeb columns in level-major dec order (see _dec_perm)
    nodeW = np.asarray(inputs["nodeW"],
                       dtype=np.float32)[DEC_T_IDX, DEC_H_IDX, :]  # [4080,128]
    nodeb = np.ascontiguousarray(np.asarray(
        inputs["nodeb"], dtype=np.float32)[DEC_T_IDX,
                                           DEC_H_IDX].reshape(1, NODES))

    XT = np.ascontiguousarray(X.T)                           # [784, 8192]
    xth, xtl = _split_bf16(XT)
    w1h, w1l = _split_bf16(W1)
    xth, xtl = _pad_k(xth), _pad_k(xtl)                      # [7, 128, 8192]
    w1h, w1l = _pad_k(w1h), _pad_k(w1l)                      # [7, 128, 128]
    nwt = np.ascontiguousarray(nodeW.T)                      # [128, 4080]
    nwh, nwl = _split_bf16(nwt)

    with_nodeb = bool(np.any(nodeb))
    with_b2 = bool(np.any(b2))

    in_maps = []
    for c in range(N_CORES):
        m = {
            "xth": np.ascontiguousarray(xth[:, :, c * BC:(c + 1) * BC]),
            "xtl": np.ascontiguousarray(xtl[:, :, c * BC:(c + 1) * BC]),
            "w1h": w1h, "w1l": w1l, "b1": b1, "w2": W2,
            "nwh": nwh, "nwl": nwl,
        }
        if with_b2:
            m["b2"] = b2
        if with_nodeb:
            m["nb"] = nodeb
        in_maps.append(m)

    nc = _get_nc(with_nodeb, with_b2)
    r = run_bass_kernel_spmd(nc, in_maps, core_ids=list(range(N_CORES)),
                             trace=trace)

    prediction = np.empty((BATCH, CLS), np.float32)
    all_results = np.empty((ENSEMBLE, TOTAL, BATCH), np.float32)
    for c in range(N_CORES):
        prediction[c * BC:(c + 1) * BC] = r.results[c]["pred"]
        # res: [NB, 128, 511, 16] (sep node positions) -> [16, 511, BC]
        rc = r.results[c]["res"].reshape(BC, TOTAL, ENSEMBLE)
        all_results[:, :, c * BC:(c + 1) * BC] = \
            rc.transpose(2, 1, 0)[:, SEP_POS, :]
    return prediction, all_results, r


def kernel(**inputs):
    prediction, all_results, _ = run_on_device(inputs, trace=False)
    return prediction, all_results
